# revision 10
# baseline (speedup 1.0000x reference)
"""
Causal self-attention (single head) on 8 trn2 NeuronCores.

Problem: x[4, 2048, 1024], Wq/Wk/Wv[1024, 1024] (torch Linear layout [d_out, d_in]).
    q/k/v = x @ W.T ; out = softmax(mask(q k^T) / 32) @ v

Sharding (no collectives, uniform SPMD program):
  core c -> batch b = c // 2, role r = c % 2.
  Both cores of a pair compute K/V projections for the full 2048-row
  sequence of their batch (duplicated work, ~11% overhead, avoids
  any cross-core communication).
  Query rows are split between the pair in 4 i-blocks of 256 rows,
  chosen so both roles see the same per-slot causal extents after
  padding to uniform j-tile counts JT_SLOTS = [4, 8, 12, 16]:
     r=0: starts [0, 768, 1024, 1792]  (actual jt 2, 8, 10, 16)
     r=1: starts [256, 512, 1280, 1536](actual jt 4, 6, 12, 14)
  Causality inside the padded slots is enforced with a per-core
  "delta" input: keep score[j, i] iff (jj - ii) <= delta(slot, t);
  delta = I0_global - 128 * t.  Only the last 4 j-tiles of each slot
  need the mask (all cores keep everything before that).

Layouts (all on-chip matmuls contract over the partition dim):
  xT   [d, s]   host-pretransposed  (k/v projections)
  xqT  [d, i_local] host-packed q-rows, pretransposed (q projection)
  WqT/WkT/WvT [d_in, d_out] host-pretransposed
  kT   [o, j] (DRAM scratch), qT [o, i_local] (DRAM scratch)
  v    [j, o] (SBUF resident, 8 MB)
  scoresT psum [j 128, i 256] = kT-tile^T @ qT-chunk  (contract o)
  expT = exp(scoresT / 32) * (T0 <= delta)            (T0[jj,ii] = jj-ii)
  ctx  psum [i 128, o 512] += expT-tile^T @ v-tile    (contract j)
  den  psum [i 128, 1]     += expT-tile^T @ ones
  out  = ctx * reciprocal(den)   (natural [i, o] layout, DMA'd out)

All matmuls run as float32r (TF32, 1 cycle/row at N>=256: 4x faster
than fp32) via bitcast views; accumulation is fp32 in PSUM.
"""

import sys

for _p in ("/opt/trn_rl_repo", "/root/.axon_site/_ro/trn_rl_repo"):
    if _p not in sys.path:
        sys.path.append(_p)

import numpy as np

import concourse.bass as bass
import concourse.mybir as mybir
import concourse.tile as tile
from concourse import bacc
from concourse.bass_utils import run_bass_kernel_spmd
import concourse.bass_utils as _bu

# walrus's --enable-ldw-opt=false leaves every LDWEIGHTS serialized with its
# MATMUL (~190 ns per MM on this kernel); enable the double-buffered
# weight-load path so LDW overlaps the previous matmul.
if not getattr(_bu, "_ldw_opt_patched", False):
    _orig_run_command = _bu.run_command

    def _run_command_ldw(cmd, *a, **kw):
        if isinstance(cmd, list):
            cmd = ["--enable-ldw-opt=true" if c == "--enable-ldw-opt=false" else c
                   for c in cmd]
        return _orig_run_command(cmd, *a, **kw)

    _bu.run_command = _run_command_ldw
    _bu._ldw_opt_patched = True

F32 = mybir.dt.float32
F32R = mybir.dt.float32r

B, S, D = 4, 2048, 1024
P = 128
ND = D // P          # 8 d-tiles (contraction tiles for projections)
NO = D // P          # 8 o-tiles
IB = 256             # i-block (query block) rows
N_IB = 4             # i-blocks per core -> 1024 query rows per core
JT_SLOTS = [4, 8, 12, 16]   # uniform j-tile count per i-block slot
N_JT = sum(JT_SLOTS)
ROLE_STARTS = {
    0: [0, 768, 1024, 1792],
    1: [256, 512, 1280, 1536],
}
N_CORES = 8
NEG = None  # masking is multiplicative (0/1), no -inf needed


def _mm(nc, out, lhsT, rhs, start, stop):
    nc.tensor.matmul(out, lhsT, rhs, start=start, stop=stop)


def build_program():
    nc = bacc.Bacc(
        "TRN2",
        target_bir_lowering=False,
        debug=False,
        enable_asserts=False,
        num_devices=N_CORES,
    )
    xT = nc.dram_tensor("xT", [D, S], F32R, kind="ExternalInput").ap()
    xqT = nc.dram_tensor("xqT", [D, N_IB * IB], F32R, kind="ExternalInput").ap()
    wqT = nc.dram_tensor("wqT", [D, D], F32R, kind="ExternalInput").ap()
    wkT = nc.dram_tensor("wkT", [D, D], F32R, kind="ExternalInput").ap()
    wvT = nc.dram_tensor("wvT", [D, D], F32R, kind="ExternalInput").ap()
    t0_in = nc.dram_tensor("t0", [P, IB], F32, kind="ExternalInput").ap()
    ones_in = nc.dram_tensor("ones", [P, 2], F32R, kind="ExternalInput").ap()
    delta_in = nc.dram_tensor("delta", [P, 16], F32, kind="ExternalInput").ap()
    out = nc.dram_tensor("out", [N_IB * IB, D], F32, kind="ExternalOutput").ap()

    scale = 1.0 / 32.0  # 1/sqrt(d_v)

    with tile.TileContext(nc) as tc:
        with (
            tc.tile_pool(name="const", bufs=1) as cpool,
            tc.tile_pool(name="vres", bufs=1) as vpool,
            tc.tile_pool(name="dram", bufs=1, space="DRAM") as dpool,
        ):
            t0_t = cpool.tile([P, IB], F32, tag="t0")
            nc.gpsimd.dma_start(t0_t[:], t0_in[:])
            delta_t = cpool.tile([P, 16], F32, tag="delta")
            nc.gpsimd.dma_start(delta_t[:], delta_in[:])
            ones_t = cpool.tile([P, 2], F32R, tag="ones")
            nc.gpsimd.dma_start(ones_t[:], ones_in[:])

            v_tiles = [
                vpool.tile([P, D], F32R, tag=f"v{j}", name=f"v{j}")
                for j in range(S // P)
            ]
            qT_dram = dpool.tile([D, N_IB * IB], F32R, tag="qTd", name="qTd")
            kT_dram = dpool.tile([D, S // 2], F32R, tag="kTd", name="kTd")
            kT_res = [
                vpool.tile([P, S // 2], F32R, tag=f"kr{o}", name=f"kr{o}")
                for o in range(NO)
            ]

            # ---------------- Phase A: projections ----------------
            # Order: q (spilled to DRAM scratch) -> v -> k, so that the
            # 16 MB of kT+v residency never coexists with the q staging.
            with (
                tc.tile_pool(name="xc", bufs=2) as xpool,
                tc.tile_pool(name="psA", bufs=2, space="PSUM") as psA,
            ):
                # PE warm-up: junk matmuls during the initial DMA window so
                # the HAM clock gate reaches 8/8 before the first real MM.
                warm = xpool.tile([P, 512], F32R, tag="warm", name="warm")
                nc.sync.dma_start(warm[:], xT[0:P, 0:512])
                wps = psA.tile([P, 512], F32, tag="wps", name="wps")
                for w in range(24):
                    _mm(nc, wps[:], warm[:, 0:P], warm[:],
                        start=True, stop=True)
                # Q projection (output spilled to qT_dram).
                # One W pool, two tag generations: wq -> even tags, wv -> odd
                # tags, wk -> even tags again (2nd generation; its loads wait
                # only on the q-stage readers, so they overlap the v stage).
                with tc.tile_pool(name="wp", bufs=1) as wpool:
                    wq_t = []
                    for d in range(ND):
                        wq = wpool.tile([P, D], F32R, tag=f"wE{d}", name=f"wq{d}")
                        nc.scalar.dma_start(wq[:], wqT[d * P:(d + 1) * P, :])
                        wq_t.append(wq)
                    wv_t = []
                    for d in range(ND):
                        wv = wpool.tile([P, D], F32R, tag=f"wO{d}", name=f"wv{d}")
                        nc.gpsimd.dma_start(wv[:], wvT[d * P:(d + 1) * P, :])
                        wv_t.append(wv)
                    with tc.tile_pool(name="stage", bufs=4) as stpool:
                        for sb in range(2):
                            xq = []
                            for d in range(ND):
                                xt = xpool.tile([P, 512], F32R, tag=f"x{d}", name=f"xq{sb}_{d}")
                                nc.sync.dma_start(
                                    xt[:], xqT[d * P:(d + 1) * P, sb * 512:(sb + 1) * 512]
                                )
                                xq.append(xt)
                            for o in range(NO):
                                pq = psA.tile([P, 512], F32, tag="pk", name=f"pq{sb}_{o}")
                                for d in range(ND):
                                    _mm(nc, pq[:], wq_t[d][:, o * P:(o + 1) * P], xq[d][:],
                                        start=(d == 0), stop=(d == ND - 1))
                                st = stpool.tile([P, 512], F32R, tag="st", name=f"stq{sb}_{o}")
                                nc.vector.tensor_copy(st[:], pq[:])
                                nc.sync.dma_start(
                                    qT_dram[o * P:(o + 1) * P, sb * 512:(sb + 1) * 512], st[:]
                                )

                    # wk: 2nd generation of the even tags
                    wk_t = []
                    for d in range(ND):
                        wk = wpool.tile([P, D], F32R, tag=f"wE{d}", name=f"wk{d}")
                        nc.gpsimd.dma_start(wk[:], wkT[d * P:(d + 1) * P, :])
                        wk_t.append(wk)

                    # V projection (stays resident in SBUF)
                    for jb in range(S // 512):
                        xv = []
                        for d in range(ND):
                            xt = xpool.tile([P, 512], F32R, tag=f"x{d}", name=f"xv{jb}_{d}")
                            nc.sync.dma_start(
                                xt[:], xT[d * P:(d + 1) * P, jb * 512:(jb + 1) * 512]
                            )
                            xv.append(xt)
                        for jj in range(4):
                            jt = jb * 4 + jj
                            for ob in range(2):
                                pv = psA.tile([P, 512], F32, tag="pv", name=f"pv{jt}_{ob}")
                                for d in range(ND):
                                    _mm(nc, pv[:],
                                        xv[d][:, jj * P:(jj + 1) * P],
                                        wv_t[d][:, ob * 512:(ob + 1) * 512],
                                        start=(d == 0), stop=(d == ND - 1))
                                nc.vector.tensor_copy(
                                    v_tiles[jt][:, ob * 512:(ob + 1) * 512], pv[:]
                                )

                    # K projection: low half resident, high half spilled
                    with tc.tile_pool(name="stage2", bufs=4) as st2pool:
                        for jb in range(S // 512):
                            xk = []
                            for d in range(ND):
                                xt = xpool.tile([P, 512], F32R, tag=f"x{d}", name=f"xk{jb}_{d}")
                                nc.sync.dma_start(
                                    xt[:], xT[d * P:(d + 1) * P, jb * 512:(jb + 1) * 512]
                                )
                                xk.append(xt)
                            for o in range(NO):
                                pk = psA.tile([P, 512], F32, tag="pk", name=f"pk{jb}_{o}")
                                for d in range(ND):
                                    _mm(nc, pk[:], wk_t[d][:, o * P:(o + 1) * P], xk[d][:],
                                        start=(d == 0), stop=(d == ND - 1))
                                if jb < 2:
                                    nc.vector.tensor_copy(
                                        kT_res[o][:, jb * 512:(jb + 1) * 512], pk[:]
                                    )
                                else:
                                    st = st2pool.tile([P, 512], F32R, tag="st2", name=f"stk{jb}_{o}")
                                    nc.vector.tensor_copy(st[:], pk[:])
                                    nc.sync.dma_start(
                                        kT_dram[o * P:(o + 1) * P,
                                                (jb - 2) * 512:(jb - 1) * 512],
                                        st[:],
                                    )

            # ---------------- Phase B: attention ----------------
            with (
                tc.tile_pool(name="kc", bufs=3) as kcpool,
                tc.tile_pool(name="qc", bufs=2) as qcpool,
                tc.tile_pool(name="ex", bufs=3) as expool,
                tc.tile_pool(name="ost", bufs=4) as ostpool,
                tc.tile_pool(name="rcp", bufs=4) as rcpool,
                tc.tile_pool(name="psS", bufs=2, space="PSUM") as psS,
                tc.tile_pool(name="psC", bufs=1, space="PSUM") as psC,
                tc.tile_pool(name="psD", bufs=1, space="PSUM") as psD,
            ):
                for s in reversed(range(N_IB)):
                    jt_n = JT_SLOTS[s]
                    # q chunk for this i-block: 8 tiles [128, 256]
                    qc = []
                    for o in range(NO):
                        q = qcpool.tile([P, IB], F32R, tag=f"qc{o}", name=f"qc{s}_{o}")
                        nc.scalar.dma_start(
                            q[:], qT_dram[o * P:(o + 1) * P, s * IB:(s + 1) * IB]
                        )
                        qc.append(q)
                    cps = [
                        [
                            psC.tile([P, 512], F32, tag=f"c{it}{ob}", name=f"c{s}_{it}{ob}")
                            for ob in range(2)
                        ]
                        for it in range(2)
                    ]
                    dps = [
                        psD.tile([P, 2], F32, tag=f"d{it}", name=f"d{s}_{it}")
                        for it in range(2)
                    ]
                    for t in range(jt_n):
                        if t >= 8:
                            kc = []
                            for o in range(NO):
                                k = kcpool.tile([P, P], F32R, tag=f"kc{o}", name=f"kc{s}_{t}_{o}")
                                nc.scalar.dma_start(
                                    k[:], kT_dram[o * P:(o + 1) * P,
                                                  (t - 8) * P:(t - 7) * P]
                                )
                                kc.append(k)
                        ps = psS.tile([P, IB], F32, tag="ps", name=f"ps{s}_{t}")
                        for o in range(NO):
                            lhsk = kc[o][:] if t >= 8 else kT_res[o][:, t * P:(t + 1) * P]
                            _mm(nc, ps[:], lhsk, qc[o][:],
                                start=(o == 0), stop=(o == NO - 1))
                        et = expool.tile([P, IB], F32R, tag="et", name=f"et{s}_{t}")
                        if t >= jt_n - 4:
                            eraw = expool.tile([P, IB], F32R, tag="eraw", name=f"er{s}_{t}")
                            nc.scalar.activation(
                                eraw[:], ps[:],
                                mybir.ActivationFunctionType.Exp, scale=scale,
                            )
                            col = s * 4 + (t - (jt_n - 4))
                            nc.vector.scalar_tensor_tensor(
                                et[:], t0_t[:], delta_t[:, col:col + 1], eraw[:],
                                op0=mybir.AluOpType.is_le,
                                op1=mybir.AluOpType.mult,
                            )
                        else:
                            nc.scalar.activation(
                                et[:], ps[:],
                                mybir.ActivationFunctionType.Exp, scale=scale,
                            )
                        last = t == jt_n - 1
                        for it in range(2):
                            lhs = et[:, it * P:(it + 1) * P]
                            for ob in range(2):
                                _mm(nc, cps[it][ob][:], lhs,
                                    v_tiles[t][:, ob * 512:(ob + 1) * 512],
                                    start=(t == 0), stop=last)
                            _mm(nc, dps[it][:], lhs, ones_t[:],
                                start=(t == 0), stop=last)
                    for it in range(2):
                        rc = rcpool.tile([P, 1], F32, tag="rc", name=f"rc{s}_{it}")
                        nc.vector.reciprocal(rc[:], dps[it][:, 0:1])
                        for ob in range(2):
                            ot = ostpool.tile([P, 512], F32, tag="ot", name=f"ot{s}_{it}{ob}")
                            nc.vector.tensor_scalar_mul(ot[:], cps[it][ob][:], rc[:])
                            nc.sync.dma_start(
                                out[s * IB + it * P: s * IB + (it + 1) * P,
                                    ob * 512:(ob + 1) * 512],
                                ot[:],
                            )

    nc.compile()
    return nc


_NC_CACHE = None


def _get_nc():
    global _NC_CACHE
    if _NC_CACHE is None:
        _NC_CACHE = build_program()
    return _NC_CACHE


def make_core_inputs(x, Wq, Wk, Wv):
    """Host-side shard prep. Returns list of 8 in_maps."""
    x = np.asarray(x, dtype=np.float32)
    wqT = np.ascontiguousarray(np.asarray(Wq, np.float32).T)
    wkT = np.ascontiguousarray(np.asarray(Wk, np.float32).T)
    wvT = np.ascontiguousarray(np.asarray(Wv, np.float32).T)
    t0 = (np.arange(P, dtype=np.float32)[:, None]
          - np.arange(IB, dtype=np.float32)[None, :])
    t0 = np.ascontiguousarray(t0)

    in_maps = []
    for c in range(N_CORES):
        b, r = divmod(c, 2)
        starts = ROLE_STARTS[r]
        xT = np.ascontiguousarray(x[b].T)
        xq = np.concatenate([x[b][i0:i0 + IB, :] for i0 in starts], axis=0)
        xqT = np.ascontiguousarray(xq.T)
        delta = np.empty((P, 16), np.float32)
        for s in range(N_IB):
            for tr in range(4):
                t = JT_SLOTS[s] - 4 + tr
                delta[:, s * 4 + tr] = float(starts[s] - P * t)
        in_maps.append({
            "xT": xT, "xqT": xqT,
            "wqT": wqT, "wkT": wkT, "wvT": wvT,
            "t0": t0, "delta": np.ascontiguousarray(delta),
            "ones": np.ones((P, 2), np.float32),
        })
    return in_maps


def assemble_output(results):
    """Gather 8 per-core [1024, 1024] outputs into [B, S, D]."""
    out = np.empty((B, S, D), np.float32)
    for c in range(N_CORES):
        b, r = divmod(c, 2)
        starts = ROLE_STARTS[r]
        oc = results[c]["out"]
        for s, i0 in enumerate(starts):
            out[b, i0:i0 + IB, :] = oc[s * IB:(s + 1) * IB, :]
    return out


def kernel(x, Wq, Wk, Wv):
    nc = _get_nc()
    in_maps = make_core_inputs(x, Wq, Wk, Wv)
    res = run_bass_kernel_spmd(nc, in_maps, list(range(N_CORES)))
    return assemble_output(res.results)


# revision 11
# speedup vs baseline: 1.1075x; 1.1075x over previous
"""
Causal self-attention (single head) on 8 trn2 NeuronCores.

Problem: x[4, 2048, 1024], Wq/Wk/Wv[1024, 1024] (torch Linear layout [d_out, d_in]).
    q/k/v = x @ W.T ; out = softmax(mask(q k^T) / 32) @ v

Sharding (no collectives, uniform SPMD program):
  core c -> batch b = c // 2, role r = c % 2.
  Both cores of a pair compute K/V projections for the full 2048-row
  sequence of their batch (duplicated work, ~26% extra PE time, avoids
  any cross-core communication).
  Query rows are split between the pair in 4 i-blocks of 256 rows,
  chosen so both roles see the same per-slot causal extents after
  padding to uniform j-tile counts JT_SLOTS = [4, 8, 12, 16]:
     r=0: starts [0, 768, 1024, 1792]  (actual jt 2, 8, 10, 16)
     r=1: starts [256, 512, 1280, 1536](actual jt 4, 6, 12, 14)
  Causality inside the padded slots is enforced with a per-core
  "delta" input: keep score[j, i] iff (jj - ii) <= delta(slot, t);
  delta = I0_global - 128 * t.  Only the last 4 j-tiles of each slot
  need the mask (earlier tiles are all-keep for both roles).

Layouts (all on-chip matmuls contract over the partition dim):
  xT   [d, s]   host-pretransposed  (k/v projections)
  xqT  [d, i_local] host-packed q-rows, pretransposed (q projection)
  WqT/WkT/WvT [d_in, d_out] host-pretransposed
  qT   [o, i_local] DRAM scratch; kT [o, j]: j<1024 SBUF-resident,
       j>=1024 DRAM scratch;  v [j, o] SBUF-resident
  scoresT psum [j 128, i 256] = kT-tile^T @ qT-chunk  (contract o)
  expT = exp(scoresT / 32) * (T0 <= delta)            (T0[jj,ii] = jj-ii)
  ctx  psum [i 128, o 512] += expT-tile^T @ v-tile    (contract j)
  den  psum [i 128, 2]     += expT-tile^T @ ones      (N=2: fp32r needs even N)
  out  = ctx * reciprocal(den)   (natural [i, o] layout, DMA'd out)

All matmuls run as float32r (TF32 mantissa, 1 cycle/row at N>=256 --
4x the plain-fp32 rate); accumulation is fp32 in PSUM.
DMAs are batched via 3D access patterns (one dma_start per 2-4 MB
chunk) because each dma_start costs ~700 ns of sequencer issue time.
"""

import sys

for _p in ("/opt/trn_rl_repo", "/root/.axon_site/_ro/trn_rl_repo"):
    if _p not in sys.path:
        sys.path.append(_p)

import numpy as np

import concourse.bass as bass
import concourse.mybir as mybir
import concourse.tile as tile
from concourse import bacc
from concourse.bass_utils import run_bass_kernel_spmd
import concourse.bass_utils as _bu

# walrus's --enable-ldw-opt=false leaves LDWEIGHTS single-buffered; enable
# the double-buffered weight-load path.
if not getattr(_bu, "_ldw_opt_patched", False):
    _orig_run_command = _bu.run_command

    def _run_command_ldw(cmd, *a, **kw):
        if isinstance(cmd, list):
            cmd = ["--enable-ldw-opt=true" if c == "--enable-ldw-opt=false" else c
                   for c in cmd]
        return _orig_run_command(cmd, *a, **kw)

    _bu.run_command = _run_command_ldw
    _bu._ldw_opt_patched = True

F32 = mybir.dt.float32
F32R = mybir.dt.float32r

B, S, D = 4, 2048, 1024
P = 128
ND = D // P          # 8 d-tiles (projection contraction)
NO = D // P          # 8 o-tiles
IB = 256             # i-block (query block) rows
N_IB = 4
JT_SLOTS = [4, 8, 12, 16]
ROLE_STARTS = {
    0: [0, 768, 1024, 1792],
    1: [256, 512, 1280, 1536],
}
N_CORES = 8


def _mm(nc, out, lhsT, rhs, start, stop):
    nc.tensor.matmul(out, lhsT, rhs, start=start, stop=stop)


def build_program():
    nc = bacc.Bacc(
        "TRN2",
        target_bir_lowering=False,
        debug=False,
        enable_asserts=False,
        num_devices=N_CORES,
    )
    xT = nc.dram_tensor("xT", [D, S], F32R, kind="ExternalInput").ap()
    xqT = nc.dram_tensor("xqT", [D, N_IB * IB], F32R, kind="ExternalInput").ap()
    wqT = nc.dram_tensor("wqT", [D, D], F32R, kind="ExternalInput").ap()
    wkT = nc.dram_tensor("wkT", [D, D], F32R, kind="ExternalInput").ap()
    wvT = nc.dram_tensor("wvT", [D, D], F32R, kind="ExternalInput").ap()
    t0_in = nc.dram_tensor("t0", [P, IB], F32, kind="ExternalInput").ap()
    delta_in = nc.dram_tensor("delta", [P, 16], F32, kind="ExternalInput").ap()
    ones_in = nc.dram_tensor("ones", [P, 2], F32R, kind="ExternalInput").ap()
    out = nc.dram_tensor("out", [N_IB * IB, D], F32, kind="ExternalOutput").ap()

    scale = 1.0 / 32.0  # 1/sqrt(d_v)

    def d_major(ap2d):
        # [ND*P, C] DRAM view -> [P, ND, C] (partition-major 3D AP)
        return ap2d.rearrange("(nd p) c -> p nd c", p=P)

    with tile.TileContext(nc) as tc:
        with (
            tc.tile_pool(name="const", bufs=1) as cpool,
            tc.tile_pool(name="vres", bufs=1) as vpool,
            tc.tile_pool(name="dram", bufs=1, space="DRAM") as dpool,
        ):
            t0_t = cpool.tile([P, IB], F32, tag="t0")
            nc.gpsimd.dma_start(t0_t[:], t0_in[:])
            delta_t = cpool.tile([P, 16], F32, tag="delta")
            nc.gpsimd.dma_start(delta_t[:], delta_in[:])
            ones_t = cpool.tile([P, 2], F32R, tag="ones")
            nc.gpsimd.dma_start(ones_t[:], ones_in[:])

            v_tiles = [
                vpool.tile([P, D], F32R, tag=f"v{j}", name=f"v{j}")
                for j in range(S // P)
            ]
            kT_res = [
                vpool.tile([P, S // 2], F32R, tag=f"kr{o}", name=f"kr{o}")
                for o in range(NO)
            ]
            qT_dram = dpool.tile([D, N_IB * IB], F32R, tag="qTd", name="qTd")
            kT_dram = dpool.tile([D, S // 2], F32R, tag="kTd", name="kTd")

            # ---------------- Phase A: projections ----------------
            with (
                tc.tile_pool(name="xc", bufs=2) as xpool,
                tc.tile_pool(name="psA", bufs=2, space="PSUM") as psA,
            ):
                # PE warm-up while the first loads land, so the HAM clock
                # gate is at 8/8 when real matmuls start.
                warm = xpool.tile([P, 512], F32R, tag="warm", name="warm")
                nc.sync.dma_start(warm[:], xT[0:P, 0:512])
                wps = psA.tile([P, 512], F32, tag="wps", name="wps")
                for w in range(24):
                    _mm(nc, wps[:], warm[:, 0:P], warm[:], start=True, stop=True)

                # One W pool, two tag generations: wq -> even, wv -> odd,
                # wk -> even again (its load overlaps the v stage).
                with tc.tile_pool(name="wp", bufs=1) as wpool:
                    wq_t = wpool.tile([P, ND, D], F32R, tag="wE", name="wq")
                    nc.scalar.dma_start(wq_t[:], d_major(wqT))
                    wv_t = wpool.tile([P, ND, D], F32R, tag="wO", name="wv")
                    nc.gpsimd.dma_start(wv_t[:], d_major(wvT))

                    # --- Q projection (spilled to qT_dram) ---
                    with tc.tile_pool(name="stage", bufs=4) as stpool:
                        for sb in range(2):
                            xq = xpool.tile([P, ND, 512], F32R, tag="xc", name=f"xq{sb}")
                            nc.sync.dma_start(
                                xq[:], d_major(xqT[:, sb * 512:(sb + 1) * 512])
                            )
                            for o in range(NO):
                                pq = psA.tile([P, 512], F32, tag="pk", name=f"pq{sb}_{o}")
                                for d in range(ND):
                                    _mm(nc, pq[:],
                                        wq_t[:, d, o * P:(o + 1) * P], xq[:, d, :],
                                        start=(d == 0), stop=(d == ND - 1))
                                st = stpool.tile([P, 512], F32R, tag="st", name=f"stq{sb}_{o}")
                                nc.vector.tensor_copy(st[:], pq[:])
                                nc.scalar.dma_start(
                                    qT_dram[o * P:(o + 1) * P, sb * 512:(sb + 1) * 512],
                                    st[:],
                                )

                    # wk: 2nd generation of the even tag
                    wk_t = wpool.tile([P, ND, D], F32R, tag="wE", name="wk")
                    nc.gpsimd.dma_start(wk_t[:], d_major(wkT))

                    # --- V projection (SBUF resident) ---
                    for jb in range(S // 512):
                        xv = xpool.tile([P, ND, 512], F32R, tag="xc", name=f"xv{jb}")
                        nc.sync.dma_start(
                            xv[:], d_major(xT[:, jb * 512:(jb + 1) * 512])
                        )
                        for jj in range(4):
                            jt = jb * 4 + jj
                            for ob in range(2):
                                pv = psA.tile([P, 512], F32, tag="pv", name=f"pv{jt}_{ob}")
                                for d in range(ND):
                                    _mm(nc, pv[:],
                                        xv[:, d, jj * P:(jj + 1) * P],
                                        wv_t[:, d, ob * 512:(ob + 1) * 512],
                                        start=(d == 0), stop=(d == ND - 1))
                                nc.vector.tensor_copy(
                                    v_tiles[jt][:, ob * 512:(ob + 1) * 512], pv[:]
                                )

                    # --- K projection (low half resident, high half spilled) ---
                    with tc.tile_pool(name="stage2", bufs=4) as st2pool:
                        for jb in range(S // 512):
                            xk = xpool.tile([P, ND, 512], F32R, tag="xc", name=f"xk{jb}")
                            nc.sync.dma_start(
                                xk[:], d_major(xT[:, jb * 512:(jb + 1) * 512])
                            )
                            for o in range(NO):
                                pk = psA.tile([P, 512], F32, tag="pk", name=f"pk{jb}_{o}")
                                for d in range(ND):
                                    _mm(nc, pk[:],
                                        wk_t[:, d, o * P:(o + 1) * P], xk[:, d, :],
                                        start=(d == 0), stop=(d == ND - 1))
                                if jb < 2:
                                    nc.vector.tensor_copy(
                                        kT_res[o][:, jb * 512:(jb + 1) * 512], pk[:]
                                    )
                                else:
                                    st = st2pool.tile([P, 512], F32R, tag="st2",
                                                      name=f"stk{jb}_{o}")
                                    nc.vector.tensor_copy(st[:], pk[:])
                                    nc.scalar.dma_start(
                                        kT_dram[o * P:(o + 1) * P,
                                                (jb - 2) * 512:(jb - 1) * 512],
                                        st[:],
                                    )

            # ---------------- Phase B: attention ----------------
            with (
                tc.tile_pool(name="kc", bufs=4) as kcpool,
                tc.tile_pool(name="qc", bufs=2) as qcpool,
                tc.tile_pool(name="ex", bufs=3) as expool,
                tc.tile_pool(name="ost", bufs=4) as ostpool,
                tc.tile_pool(name="rcp", bufs=4) as rcpool,
                tc.tile_pool(name="psS", bufs=2, space="PSUM") as psS,
                tc.tile_pool(name="psC", bufs=1, space="PSUM") as psC,
                tc.tile_pool(name="psD", bufs=1, space="PSUM") as psD,
            ):
                for s in reversed(range(N_IB)):
                    jt_n = JT_SLOTS[s]
                    qc = qcpool.tile([P, NO, IB], F32R, tag="qc", name=f"qc{s}")
                    nc.scalar.dma_start(
                        qc[:], d_major(qT_dram[:, s * IB:(s + 1) * IB])
                    )
                    cps = [
                        [
                            psC.tile([P, 512], F32, tag=f"c{it}{ob}", name=f"c{s}_{it}{ob}")
                            for ob in range(2)
                        ]
                        for it in range(2)
                    ]
                    dps = [
                        psD.tile([P, 2], F32, tag=f"d{it}", name=f"d{s}_{it}")
                        for it in range(2)
                    ]
                    for t in range(jt_n):
                        if t >= 8:
                            kc = kcpool.tile([P, NO, P], F32R, tag="kc", name=f"kc{s}_{t}")
                            nc.scalar.dma_start(
                                kc[:], d_major(kT_dram[:, (t - 8) * P:(t - 7) * P])
                            )
                        ps = psS.tile([P, IB], F32, tag="ps", name=f"ps{s}_{t}")
                        for o in range(NO):
                            lhsk = (kc[:, o, :] if t >= 8
                                    else kT_res[o][:, t * P:(t + 1) * P])
                            _mm(nc, ps[:], lhsk, qc[:, o, :],
                                start=(o == 0), stop=(o == NO - 1))
                        et = expool.tile([P, IB], F32R, tag="et", name=f"et{s}_{t}")
                        if t >= jt_n - 4:
                            eraw = expool.tile([P, IB], F32R, tag="eraw", name=f"er{s}_{t}")
                            nc.scalar.activation(
                                eraw[:], ps[:],
                                mybir.ActivationFunctionType.Exp, scale=scale,
                            )
                            col = s * 4 + (t - (jt_n - 4))
                            nc.vector.scalar_tensor_tensor(
                                et[:], t0_t[:], delta_t[:, col:col + 1], eraw[:],
                                op0=mybir.AluOpType.is_le,
                                op1=mybir.AluOpType.mult,
                            )
                        else:
                            nc.scalar.activation(
                                et[:], ps[:],
                                mybir.ActivationFunctionType.Exp, scale=scale,
                            )
                        last = t == jt_n - 1
                        for it in range(2):
                            lhs = et[:, it * P:(it + 1) * P]
                            for ob in range(2):
                                _mm(nc, cps[it][ob][:], lhs,
                                    v_tiles[t][:, ob * 512:(ob + 1) * 512],
                                    start=(t == 0), stop=last)
                            _mm(nc, dps[it][:], lhs, ones_t[:],
                                start=(t == 0), stop=last)
                    for it in range(2):
                        rc = rcpool.tile([P, 1], F32, tag="rc", name=f"rc{s}_{it}")
                        nc.vector.reciprocal(rc[:], dps[it][:, 0:1])
                        ot = ostpool.tile([P, D], F32, tag="ot", name=f"ot{s}_{it}")
                        for ob in range(2):
                            nc.vector.tensor_scalar_mul(
                                ot[:, ob * 512:(ob + 1) * 512], cps[it][ob][:], rc[:]
                            )
                        nc.sync.dma_start(
                            out[s * IB + it * P: s * IB + (it + 1) * P, :], ot[:]
                        )

    nc.compile()
    return nc


_NC_CACHE = None


def _get_nc():
    global _NC_CACHE
    if _NC_CACHE is None:
        _NC_CACHE = build_program()
    return _NC_CACHE


def make_core_inputs(x, Wq, Wk, Wv):
    """Host-side shard prep. Returns list of 8 in_maps."""
    x = np.asarray(x, dtype=np.float32)
    wqT = np.ascontiguousarray(np.asarray(Wq, np.float32).T)
    wkT = np.ascontiguousarray(np.asarray(Wk, np.float32).T)
    wvT = np.ascontiguousarray(np.asarray(Wv, np.float32).T)
    t0 = (np.arange(P, dtype=np.float32)[:, None]
          - np.arange(IB, dtype=np.float32)[None, :])
    t0 = np.ascontiguousarray(t0)

    in_maps = []
    for c in range(N_CORES):
        b, r = divmod(c, 2)
        starts = ROLE_STARTS[r]
        xT = np.ascontiguousarray(x[b].T)
        xq = np.concatenate([x[b][i0:i0 + IB, :] for i0 in starts], axis=0)
        xqT = np.ascontiguousarray(xq.T)
        delta = np.empty((P, 16), np.float32)
        for s in range(N_IB):
            for tr in range(4):
                t = JT_SLOTS[s] - 4 + tr
                delta[:, s * 4 + tr] = float(starts[s] - P * t)
        in_maps.append({
            "xT": xT, "xqT": xqT,
            "wqT": wqT, "wkT": wkT, "wvT": wvT,
            "t0": t0, "delta": np.ascontiguousarray(delta),
            "ones": np.ones((P, 2), np.float32),
        })
    return in_maps


def assemble_output(results):
    """Gather 8 per-core [1024, 1024] outputs into [B, S, D]."""
    out = np.empty((B, S, D), np.float32)
    for c in range(N_CORES):
        b, r = divmod(c, 2)
        starts = ROLE_STARTS[r]
        oc = results[c]["out"]
        for s, i0 in enumerate(starts):
            out[b, i0:i0 + IB, :] = oc[s * IB:(s + 1) * IB, :]
    return out


def kernel(x, Wq, Wk, Wv):
    nc = _get_nc()
    in_maps = make_core_inputs(x, Wq, Wk, Wv)
    res = run_bass_kernel_spmd(nc, in_maps, list(range(N_CORES)))
    return assemble_output(res.results)
